# revision 21
# baseline (speedup 1.0000x reference)
"""Trainium2 Bass kernel for a dense transformer block (causal attn + MLP).

Problem: B=4, L=2048, D=1024, H=16 (DH=64), DFF=4096, fp32 in/out.

Sharding (no collectives): 8 cores = 4 batches x 2 parity groups.
Core c handles batch b=c//2 and query-row tiles {p, p+2, ..., p+14}
(p=c%2). The host hands each core its batch TRANSPOSED (feature-major)
and PERMUTED: own sequence tiles first [0:1024), the pair's tiles after
[1024:2048). In permuted space the causal structure is parity-uniform:
own-key tiles hit a STATIC 128x128 triangle mask exactly on the
diagonal chunk, other-key tiles need only an all-0/all-1 per-parity
constant on their boundary chunk - no per-tile mask tables, and Q/K/V,
attention, and the MLP all address compile-time column ranges.

LayerNorm1 runs directly in the transposed layout: an all-1/D bf16
stationary matrix makes the PE stats matmuls produce mean / E[x^2]
pre-broadcast to all 128 partitions (no transposes, no single-partition
ops); ACT squares feed the second-moment accumulation. Q/K/V
projections run as fp8 DoubleRow matmuls (weights + normalized
activations in fp8e4). Scores stay bf16 (K^T/Q^T tiles); exp/V/AV run
in fp8. Softmax normalization is deferred through the AV matmul via
ones columns packed into V; 1/sumexp comes from one fast-approx DVE
reciprocal over the packed sumexp row and a rank-1 PE broadcast. AV
stays in SBUF (bf16) straight into the WO matmul. LN2 reuses the
spread-stats trick, merged into the WO/residual loop. W1 prefetches
during C/D.

dtypes: fp8e4 for QKV weights+activations, V, exp; bf16 for K^T/Q^T,
AV, WO/W1/W2, h, LN2-normalized activations; residual stream fp32(r).
"""

import numpy as np
import ml_dtypes

import concourse.bacc as bacc
import concourse.bass as bass
import concourse.mybir as mybir
import concourse.tile as tile
from concourse.bass_utils import run_bass_kernel_spmd

F32 = mybir.dt.float32
F32R = mybir.dt.float32r
BF16 = mybir.dt.bfloat16
FP8 = mybir.dt.float8e4
BF = ml_dtypes.bfloat16
F8 = ml_dtypes.float8_e4m3fn
EPS = 1e-5
AF = mybir.ActivationFunctionType
OP = mybir.AluOpType
DRM = mybir.MatmulPerfMode.DoubleRow

B_, L_, D_, H_, DFF_ = 4, 2048, 1024, 16, 4096
N_CORES = 8


def _derived(L, D, H, DFF):
    CT = D // 128
    FT = DFF // 128
    n_lt = L // 128
    n_own = n_lt // 2
    OWN_L = n_own * 128
    assert n_own % 4 == 0
    NB = n_own // 4
    HT = H // 2
    assert CT == HT
    VW = min(512, D)
    return dict(CT=CT, FT=FT, n_lt=n_lt, n_own=n_own, OWN_L=OWN_L, NB=NB,
                HT=HT, VW=VW, DVB=D // VW)


def build_nc(L=L_, D=D_, H=H_, DFF=DFF_, n_cores=N_CORES):
    g = _derived(L, D, H, DFF)
    CT, FT = g["CT"], g["FT"]
    n_lt, n_own, OWN_L = g["n_lt"], g["n_own"], g["OWN_L"]
    NB = g["NB"]
    HT, VW, DVB = g["HT"], g["VW"], g["DVB"]
    W = 512
    NBLK = L // W  # A-phase column blocks (permuted; blocks 0..NBLK/2-1 own)
    scale = 1.0 / 8.0  # 1/sqrt(DH)

    nc = bacc.Bacc("TRN2", target_bir_lowering=False, debug=False,
                   num_devices=n_cores)

    dp = nc.declare_dram_parameter
    xt_d = dp("xt", [128, CT, L], BF16, isOutput=False)     # X^T permuted
    xot_d = dp("xot", [128, CT, OWN_L], F32, isOutput=False)  # own X^T exact
    wq_d = dp("wq", [128, CT, CT, 128], FP8, isOutput=False)  # [p, d, c, q]
    wk_d = dp("wk", [128, CT, CT, 128], FP8, isOutput=False)
    wv_d = dp("wv", [128, CT, D], BF16, isOutput=False)        # [p, c, dv]
    wo_d = dp("wo", [CT, 128, CT, 128], BF16, isOutput=False)  # [e, p, c, q]
    w1_d = dp("w1", [FT, 128, CT, 128], BF16, isOutput=False)  # [f, p, c, q]
    w2_d = dp("w2", [CT, 128, FT, 128], BF16, isOutput=False)  # [e, p, f, q]
    bq_d = dp("bqc", [128, CT], F32, isOutput=False)
    bk_d = dp("bkc", [128, CT], F32, isOutput=False)
    b1_d = dp("b1c", [128, FT], F32, isOutput=False)
    boeff_d = dp("boeffc", [128, CT], F32, isOutput=False)
    b2_d = dp("b2c", [128, CT], F32, isOutput=False)
    onesrv_d = dp("onesrv", [1, 128], F32, isOutput=False)
    invdrv_d = dp("invdrv", [1, 128], F32, isOutput=False)
    tri_d = dp("tri", [128, 2, 128], BF16, isOutput=False)
    pblk_d = dp("pblk", [128, 2, 128], BF16, isOutput=False)
    out_d = dp("outT", [D, OWN_L], F32, isOutput=True)

    with tile.TileContext(nc) as tc, \
         nc.allow_low_precision(reason="fp8/f32r/bf16 matmul operands"):
        consts_cm = tc.tile_pool(name="consts", bufs=1)
        consts = consts_cm.__enter__()

        eps_c = consts.tile([128, 1], F32, tag="eps")
        nc.vector.memset(eps_c[:], EPS)
        invd_bf = consts.tile([128, 128], BF16, tag="invdbf")
        nc.vector.memset(invd_bf[:], 1.0 / D)
        _iap = invdrv_d[:]
        invd_m = consts.tile([128, 128], F32R, tag="invdm")
        nc.sync.dma_start(out=invd_m[:], in_=bass.AP(
            tensor=_iap.tensor, offset=_iap.offset,
            ap=[[0, 128], [1, 128]]).bitcast(F32R))
        _oap = onesrv_d[:]
        onesm = consts.tile([128, 128], F32R, tag="onesm")
        nc.sync.dma_start(out=onesm[:], in_=bass.AP(
            tensor=_oap.tensor, offset=_oap.offset,
            ap=[[0, 128], [1, 128]]).bitcast(F32R))
        tri2 = consts.tile([128, 2, 128], BF16, tag="tri2")
        nc.sync.dma_start(out=tri2[:], in_=tri_d[:])
        pblk2 = consts.tile([128, 2, 128], BF16, tag="pblk2")
        nc.sync.dma_start(out=pblk2[:], in_=pblk_d[:])
        bq_sb = consts.tile([128, CT], F32, tag="bq")
        nc.sync.dma_start(out=bq_sb[:], in_=bq_d[:])
        bk_sb = consts.tile([128, CT], F32, tag="bk")
        nc.sync.dma_start(out=bk_sb[:], in_=bk_d[:])
        b1_sb = consts.tile([128, FT], F32, tag="b1")
        nc.sync.dma_start(out=b1_sb[:], in_=b1_d[:])
        boeff_sb = consts.tile([128, CT], F32, tag="boeff")
        nc.sync.dma_start(out=boeff_sb[:], in_=boeff_d[:])
        b2_sb = consts.tile([128, CT], F32, tag="b2")
        nc.sync.dma_start(out=b2_sb[:], in_=b2_d[:])

        # avsb spans B..C (LIFO pool order: open early)
        avsb_cm = tc.tile_pool(name="avsb", bufs=1)
        p_avsb = avsb_cm.__enter__()
        avsb = [p_avsb.tile([128, OWN_L], BF16, tag=f"av{i}", name=f"av{i}")
                for i in range(CT)]
        woP_cm = tc.tile_pool(name="woP", bufs=1)
        woP = woP_cm.__enter__()
        wo_sb = []
        for ei in range(CT):
            wt = woP.tile([128, CT, 128], BF16, tag=f"wo_lhsT{ei}",
                          name=f"wo_lhsT{ei}")
            nc.sync.dma_start(out=wt[:], in_=wo_d[ei])
            wo_sb.append(wt)

        attio_cm = tc.tile_pool(name="attio", bufs=1)
        attio = attio_cm.__enter__()
        kt = [attio.tile([128, L], BF16, tag=f"kt{i}", name=f"kt{i}")
              for i in range(CT)]
        qt = [attio.tile([128, OWN_L], BF16, tag=f"qt{i}", name=f"qt{i}")
              for i in range(CT)]
        v1 = attio.tile([128, n_lt, H, 65], BF16, tag="v", name="v1")
        nc.vector.memset(v1[:], 1.0)

        # ======== Phase A (LN1+V) then fused per-head K/Q + attention ====
        xn8p_cm = tc.tile_pool(name="xn8p", bufs=1)
        xn8p = xn8p_cm.__enter__()
        xn8s = [xn8p.tile([128, CT, W], FP8, tag=f"xn8_{b}",
                          name=f"xn8_{b}") for b in range(NBLK)]
        wvP_cm = tc.tile_pool(name="wvP", bufs=1)
        wvP = wvP_cm.__enter__()
        wv_sb = wvP.tile([128, CT, D], BF16, tag="wv", name="wv_sb")
        nc.sync.dma_start(out=wv_sb[:], in_=wv_d[:])
        wA_cm = tc.tile_pool(name="workA", bufs=2)
        wA = wA_cm.__enter__()
        psA_st_cm = tc.tile_pool(name="psA_st", bufs=2, space="PSUM")
        psA_st = psA_st_cm.__enter__()
        psA_mm_cm = tc.tile_pool(name="psA_mm", bufs=4, space="PSUM")
        psA_mm = psA_mm_cm.__enter__()

        def ln_blk(blk):
            """LN1 of one 512-col block, fully in the transposed layout.
            Stats matmuls with the all-1/D stationary operand produce
            mean / E[x^2] broadcast to every partition."""
            xt = wA.tile([128, CT, W], BF16, tag="xt", bufs=2)
            nc.sync.dma_start(out=xt[:],
                              in_=xt_d[:, :, blk * W:(blk + 1) * W])
            sq_all = wA.tile([128, CT, W], BF16, tag="sq", bufs=2)
            nc.scalar.activation(out=sq_all[:], in_=xt[:], func=AF.Square)
            ps_mu = psA_st.tile([128, W], F32, tag="ps_mu")
            ps_sq = psA_st.tile([128, W], F32, tag="ps_sq")
            for ci in range(CT):
                nc.tensor.matmul(ps_mu[:], invd_bf[:], xt[:, ci, :],
                                 start=(ci == 0), stop=(ci == CT - 1))
            for ci in range(CT):
                nc.tensor.matmul(ps_sq[:], invd_bf[:], sq_all[:, ci, :],
                                 start=(ci == 0), stop=(ci == CT - 1))
            mu2 = wA.tile([128, W], F32, tag="mu2", bufs=1)
            nc.vector.tensor_copy(out=mu2[:], in_=ps_mu[:])
            varr = wA.tile([128, W], F32, tag="varr", bufs=1)
            nc.vector.tensor_mul(varr[:], mu2[:], mu2[:])
            nc.vector.tensor_sub(varr[:], ps_sq[:], varr[:])
            std = wA.tile([128, W], F32, tag="std", bufs=1)
            nc.scalar.activation(out=std[:], in_=varr[:], func=AF.Sqrt,
                                 bias=eps_c[:])
            rstd = wA.tile([128, W], F32, tag="rstd", bufs=1)
            nc.vector.reciprocal_approx_fast(out=rstd[:], in_=std[:])
            xnb = wA.tile([128, CT, W], BF16, tag="xnb")
            for ci in range(CT):
                t1 = wA.tile([128, W], F32, tag="t1")
                nc.vector.tensor_sub(t1[:], xt[:, ci, :], ps_mu[:])
                nc.vector.tensor_mul(xnb[:, ci, :], t1[:], rstd[:])
                nc.vector.tensor_copy(out=xn8s[blk][:, ci, :],
                                      in_=xnb[:, ci, :])
            return xnb

        for blk in range(NBLK):
            xnb = ln_blk(blk)
            for st4 in range(4):
                st = 4 * blk + st4
                for vb in range(DVB):
                    ps = psA_mm.tile([128, VW], F32, tag="ps_mm")
                    for ci in range(CT):
                        nc.tensor.matmul(
                            ps[:],
                            xnb[:, ci, st4 * 128:(st4 + 1) * 128],
                            wv_sb[:, ci, vb * VW:(vb + 1) * VW],
                            start=(ci == 0), stop=(ci == CT - 1))
                    nhh = VW // 64
                    nc.vector.tensor_copy(
                        out=v1[:, st, vb * nhh:(vb + 1) * nhh, 0:64],
                        in_=ps[:].rearrange("p (h d) -> p h d", d=64))

        for cm in (psA_mm_cm, psA_st_cm, wA_cm, wvP_cm):
            cm.__exit__(None, None, None)

        # ====== fused per-head-pair K/Q projection + attention ==========
        wqkv_cm = tc.tile_pool(name="wqkv", bufs=1)
        wqkv = wqkv_cm.__enter__()
        wq_sb = wqkv.tile([128, CT, CT, 128], FP8, tag="wq", name="wq_sb")
        nc.sync.dma_start(out=wq_sb[:], in_=wq_d[:])
        wk_sb = wqkv.tile([128, CT, CT, 128], FP8, tag="wk", name="wk_sb")
        nc.sync.dma_start(out=wk_sb[:], in_=wk_d[:])
        wB_cm = tc.tile_pool(name="workB", bufs=4)
        wB = wB_cm.__enter__()
        wR_cm = tc.tile_pool(name="rec", bufs=2)
        wR = wR_cm.__enter__()
        psB_sc_cm = tc.tile_pool(name="psB_sc", bufs=2, space="PSUM")
        psB_sc = psB_sc_cm.__enter__()
        psB_av_cm = tc.tile_pool(name="psB_av", bufs=2, space="PSUM")
        psB_av = psB_av_cm.__enter__()

        def norm_tail(st):
            """Softmax-normalize block (ht,Bk): fast-approx reciprocal of
            the packed sumexp row, PE rank-1 broadcast, DVE multiply into
            avsb. Emitted during the NEXT (ht,Bk) iteration so its latency
            chain never stalls the PE."""
            t_ht, t_Bk, t_av = st
            se = wR.tile([65, 2 * W], F32R, tag="se", name="se")
            for hp in range(2):
                nc.vector.tensor_copy(out=se[64:65, hp * W:(hp + 1) * W],
                                      in_=t_av[hp][64:65, :])
            for hp in range(2):
                ps_bc = psB_sc.tile([128, 2 * W], F32, tag="ps_sc",
                                    name="bc")
                nc.tensor.matmul(ps_bc[0:64, 0:W],
                                 onesm[64:65, 0:64],
                                 se[64:65, hp * W:(hp + 1) * W],
                                 start=True, stop=True)
                bcse = wR.tile([64, W], F32, tag=f"bcse{hp}",
                               name=f"bcse{hp}")
                nc.vector.tensor_copy(out=bcse[:], in_=ps_bc[0:64, 0:W])
                recb = wR.tile([64, W], F32, tag=f"recb{hp}",
                               name=f"recb{hp}")
                nc.vector.reciprocal_approx_fast(out=recb[:], in_=bcse[:])
                nc.vector.tensor_mul(
                    avsb[t_ht][64 * hp:64 * hp + 64,
                               t_Bk * W:(t_Bk + 1) * W],
                    t_av[hp][0:64, :], recb[:])

        def kq_for_ht(ht):
            """K^T (all 4 blocks) and Q^T (2 own blocks) for head pair ht,
            emitted right before this head pair's attention. PSUM comes
            from the score-tile rotation."""
            for qk, wsb, nblk, ktile, bias in (
                    (0, wk_sb, NBLK, kt[ht], bk_sb),
                    (1, wq_sb, NBLK // 2, qt[ht], bq_sb)):
                for blk in range(nblk):
                    half = blk % 2
                    if half == 0:
                        kq_ps = psB_sc.tile([128, 2 * W], F32, tag="ps_sc",
                                            name="kq")
                    for c2 in range(CT // 2):
                        nc.tensor.matmul(
                            kq_ps[:, half * W:(half + 1) * W],
                            wsb[:, ht, 2 * c2:2 * c2 + 2, :],
                            xn8s[blk][:, 2 * c2:2 * c2 + 2, :],
                            perf_mode=DRM, start=(c2 == 0),
                            stop=(c2 == CT // 2 - 1))
                    nc.vector.tensor_scalar_add(
                        out=ktile[:, blk * W:(blk + 1) * W],
                        in0=kq_ps[:, half * W:(half + 1) * W],
                        scalar1=bias[:, ht:ht + 1])

        pending = None
        for ht in range(HT):
            kq_for_ht(ht)
            for Bk in range(NB):
                nt = 4 * Bk + 4
                units = []
                for t in range(nt):  # own-group key tiles
                    units.append((t, 128 * max(0, t - 4 * Bk),
                                  tri2 if t >= 4 * Bk else None))
                for t in range(nt):  # other-group key tiles
                    units.append((8 + t, 128 * max(0, t - 4 * Bk),
                                  pblk2 if t >= 4 * Bk else None))
                n_s = len(units)
                ps_av = [psB_av.tile([128, W], F32, tag=f"ps_av{hp}",
                                     name=f"ps_av{hp}") for hp in range(2)]

                def av_pair(ui, pi, pex, plo):
                    for hp in range(2):
                        nc.tensor.matmul(
                            ps_av[hp][0:65, plo:W],
                            v1[:, pi, 2 * ht + hp, :],
                            pex[:, hp * W + plo:(hp + 1) * W],
                            start=(ui == 0), stop=(ui == n_s - 1))

                prevs = []
                for ui, (pi, lo, mk) in enumerate(units):
                    ps_sc = psB_sc.tile([128, 2 * W], F32, tag="ps_sc",
                                        name="ps_sc")
                    for hp in range(2):
                        nc.tensor.matmul(
                            ps_sc[:, hp * W + lo:(hp + 1) * W],
                            kt[ht][64 * hp:64 * hp + 64,
                                   pi * 128:(pi + 1) * 128],
                            qt[ht][64 * hp:64 * hp + 64,
                                   Bk * W + lo:(Bk + 1) * W],
                            start=True, stop=True)
                    ex = wB.tile([128, 2 * W], BF16, tag="exp", name="ex")
                    ex3 = ex[:].rearrange("p (h w) -> p h w", h=2)
                    nc.scalar.activation(
                        out=ex3[:, :, lo:W],
                        in_=ps_sc[:].rearrange("p (h w) -> p h w",
                                               h=2)[:, :, lo:W],
                        func=AF.Exp, scale=scale)
                    if mk is not None:
                        nc.vector.tensor_mul(ex3[:, :, lo:lo + 128],
                                             ex3[:, :, lo:lo + 128],
                                             mk[:])
                    # two-deep software pipeline: AV of unit ui-2 runs on
                    # the PE while exp/mask of ui-1/ui are still on ACT/DVE
                    prevs.append((ui, pi, ex, lo))
                    if len(prevs) > 3:
                        av_pair(*prevs.pop(0))
                    if ui == 2 and pending is not None:
                        norm_tail(pending)
                        pending = None
                for pv in prevs:
                    av_pair(*pv)
                pending = (ht, Bk, ps_av)
        norm_tail(pending)

        for cm in (psB_av_cm, psB_sc_cm, wR_cm, wB_cm, wqkv_cm, xn8p_cm):
            cm.__exit__(None, None, None)
        attio_cm.__exit__(None, None, None)

        # ====== long-lived pools open early: x2t (C..E), x2nt (D->E) and
        # the W1 prefetch ring (DMA spans C/D) =======
        x2t_cm = tc.tile_pool(name="x2t", bufs=1)
        p_x2t = x2t_cm.__enter__()
        x2t = [p_x2t.tile([128, OWN_L], F32R, tag=f"x2t{i}", name=f"x2t{i}")
               for i in range(CT)]
        x2nt_cm = tc.tile_pool(name="x2nt", bufs=1)
        p_x2nt = x2nt_cm.__enter__()
        x2nt = [p_x2nt.tile([128, OWN_L], BF16, tag=f"x2nt{i}",
                            name=f"x2nt{i}") for i in range(CT)]
        wE_cm = tc.tile_pool(name="workE", bufs=2)
        wE = wE_cm.__enter__()
        w1p_cm = tc.tile_pool(name="w1p", bufs=7)
        w1p = w1p_cm.__enter__()

        NPRE = 7
        w1t = {}
        for f in range(NPRE):
            wt = w1p.tile([128, CT, 128], BF16, tag="w1_lhsT")
            nc.sync.dma_start(out=wt[:], in_=w1_d[f])
            w1t[f] = wt

        # ========= Phase C+D: WO proj + residual + LN2 (merged) =========
        wD_cm = tc.tile_pool(name="workD", bufs=1)
        wD = wD_cm.__enter__()
        psD_cm = tc.tile_pool(name="psD", bufs=1, space="PSUM")
        psD = psD_cm.__enter__()
        xtC_cm = tc.tile_pool(name="xtC", bufs=1)
        p_xtC = xtC_cm.__enter__()
        xt_own = [p_xtC.tile([128, OWN_L], F32, tag=f"xto{i}", name=f"xto{i}")
                  for i in range(CT)]
        psC_cm = tc.tile_pool(name="psC", bufs=3, space="PSUM")
        psC = psC_cm.__enter__()

        for ci in range(CT):
            nc.sync.dma_start(out=xt_own[ci][:], in_=xot_d[:, ci, :])

        def phase_c(nb):
            for ei in range(CT):
                ps = psC.tile([128, W], F32, tag="ps_o")
                for ci in range(CT):
                    nc.tensor.matmul(ps[:], wo_sb[ei][:, ci, :],
                                     avsb[ci][:, nb * W:(nb + 1) * W],
                                     start=(ci == 0), stop=(ci == CT - 1))
                nc.vector.scalar_tensor_tensor(
                    out=x2t[ei][:, nb * W:(nb + 1) * W], in0=ps[:],
                    scalar=boeff_sb[:, ei:ei + 1],
                    in1=xt_own[ei][:, nb * W:(nb + 1) * W],
                    op0=OP.add, op1=OP.add)

        def phase_d_sq(nb):
            """ACT squares for LN2 stats of block nb (emitted early so the
            PE never waits on them)."""
            sqs = []
            for ci in range(CT):
                sq = wD.tile([128, W], F32R, tag="sq", bufs=8)
                nc.scalar.activation(out=sq[:],
                                     in_=x2t[ci][:, nb * W:(nb + 1) * W],
                                     func=AF.Square)
                sqs.append(sq)
            return sqs

        def phase_d(nb, sqs):
            """LN2 of block nb in transposed layout (spread-stats trick)."""
            ps_mu = psD.tile([128, W], F32, tag="ps_mu")
            ps_sq = psD.tile([128, W], F32, tag="ps_sq")
            for ci in range(CT):
                nc.tensor.matmul(ps_mu[:], invd_m[:],
                                 x2t[ci][:, nb * W:(nb + 1) * W],
                                 start=(ci == 0), stop=(ci == CT - 1))
            for ci in range(CT):
                nc.tensor.matmul(ps_sq[:], invd_m[:], sqs[ci][:],
                                 start=(ci == 0), stop=(ci == CT - 1))
            mu2 = wD.tile([128, W], F32, tag="mu2")
            nc.vector.tensor_copy(out=mu2[:], in_=ps_mu[:])
            varr = wD.tile([128, W], F32, tag="varr")
            nc.vector.tensor_mul(varr[:], mu2[:], mu2[:])
            nc.vector.tensor_sub(varr[:], ps_sq[:], varr[:])
            std = wD.tile([128, W], F32, tag="std")
            nc.scalar.activation(out=std[:], in_=varr[:], func=AF.Sqrt,
                                 bias=eps_c[:])
            rstd = wD.tile([128, W], F32, tag="rstd")
            nc.vector.reciprocal_approx_fast(out=rstd[:], in_=std[:])
            for ci in range(CT):
                t1 = wD.tile([128, W], F32, tag="t1", bufs=1)
                nc.vector.tensor_sub(t1[:], x2t[ci][:, nb * W:(nb + 1) * W],
                                     ps_mu[:])
                nc.vector.tensor_mul(x2nt[ci][:, nb * W:(nb + 1) * W],
                                     t1[:], rstd[:])

        phase_c(0)
        sqs0 = phase_d_sq(0)
        phase_c(1)
        phase_d(0, sqs0)
        sqs1 = phase_d_sq(1)
        phase_d(1, sqs1)

        for cm in (psD_cm, psC_cm, wD_cm, wC_cm, xtC_cm):
            cm.__exit__(None, None, None)

        # ================= Phase E: MLP =================================
        ht_cm = tc.tile_pool(name="hpool", bufs=1)
        p_ht = ht_cm.__enter__()
        h_sb = [p_ht.tile([128, OWN_L], BF16, tag=f"h{i}", name=f"h{i}")
                for i in range(FT)]
        psE_cm = tc.tile_pool(name="psE", bufs=3, space="PSUM")
        psE = psE_cm.__enter__()

        for f in range(FT):
            if f in w1t:
                wtile = w1t.pop(f)
            else:
                wtile = w1p.tile([128, CT, 128], BF16, tag="w1_lhsT")
                nc.sync.dma_start(out=wtile[:], in_=w1_d[f])
            for nb in range(NB):
                ps = psE.tile([128, W], F32, tag="ps_h")
                for ci in range(CT):
                    nc.tensor.matmul(ps[:], wtile[:, ci, :],
                                     x2nt[ci][:, nb * W:(nb + 1) * W],
                                     start=(ci == 0), stop=(ci == CT - 1))
                nc.scalar.activation(out=h_sb[f][:, nb * W:(nb + 1) * W],
                                     in_=ps[:], func=AF.Relu,
                                     bias=b1_sb[:, f:f + 1])
        for ei in range(CT):
            wtile = wE.tile([128, FT, 128], BF16, tag="w2_lhsT")
            nc.sync.dma_start(out=wtile[:], in_=w2_d[ei])
            for nb in range(NB):
                ps = psE.tile([128, W], F32, tag="ps_o2")
                for f in range(FT):
                    nc.tensor.matmul(ps[:], wtile[:, f, :],
                                     h_sb[f][:, nb * W:(nb + 1) * W],
                                     start=(f == 0), stop=(f == FT - 1))
                osb = wE.tile([128, W], F32, tag="osb")
                nc.vector.scalar_tensor_tensor(
                    out=osb[:], in0=ps[:], scalar=b2_sb[:, ei:ei + 1],
                    in1=x2t[ei][:, nb * W:(nb + 1) * W],
                    op0=OP.add, op1=OP.add)
                nc.sync.dma_start(
                    out=out_d[ei * 128:(ei + 1) * 128, nb * W:(nb + 1) * W],
                    in_=osb[:])

        for cm in (psE_cm, ht_cm, psD_cm, wD_cm, w1p_cm, wE_cm, x2nt_cm,
                   x2t_cm, woP_cm, avsb_cm, consts_cm):
            cm.__exit__(None, None, None)

    nc.compile()
    return nc, g


def _tile_lhsT(wmat):
    """[K, M] -> [m, p, c, q] with out[m, p, c, q] = wmat[128c+p, 128m+q]."""
    K, M = wmat.shape
    CT, MT = K // 128, M // 128
    w = wmat.reshape(CT, 128, MT, 128)
    return np.ascontiguousarray(w.transpose(2, 1, 0, 3))


def _fp8(a):
    return np.clip(a, -240.0, 240.0).astype(F8)


def _xT_tiles(Xb, CT):
    """[L?, D] row-major -> [128, CT, L?] feature-major tiles."""
    T = np.ascontiguousarray(Xb.T)  # [D, L]
    Dd, Ln = T.shape
    return np.ascontiguousarray(T.reshape(CT, 128, Ln).transpose(1, 0, 2))


def prep_in_maps(inputs, L=L_, D=D_, H=H_, DFF=DFF_, Bn=B_):
    f64 = lambda k: np.asarray(inputs[k], np.float64)
    X = np.asarray(inputs["X"], np.float32)
    WQ, WK, WV, WO = f64("WQ"), f64("WK"), f64("WV"), f64("WO")
    W1, W2 = f64("W1"), f64("W2")
    bQ, bK, bV, bO = f64("bQ"), f64("bK"), f64("bV"), f64("bO")
    b1, b2 = f64("b1"), f64("b2")
    g1, be1, g2, be2 = f64("g1"), f64("be1"), f64("g2"), f64("be2")

    g = _derived(L, D, H, DFF)
    CT, FT, n_own = g["CT"], g["FT"], g["n_own"]

    # fold LayerNorm affine transforms into the downstream weights
    WQf, bQf = g1[:, None] * WQ, bQ + be1 @ WQ
    WKf, bKf = g1[:, None] * WK, bK + be1 @ WK
    WVf, bVf = g1[:, None] * WV, bV + be1 @ WV
    boeff = bO + WO.T @ bVf
    W1f, b1f = g2[:, None] * W1, b1 + be2 @ W1

    c32 = lambda a: np.ascontiguousarray(a).astype(np.float32)
    wq_t = _fp8(np.ascontiguousarray(
        _tile_lhsT(WQf).transpose(1, 0, 2, 3)))
    wk_t = _fp8(np.ascontiguousarray(
        _tile_lhsT(WKf).transpose(1, 0, 2, 3)))
    wv_r = np.ascontiguousarray(
        WVf.reshape(CT, 128, D).transpose(1, 0, 2)).astype(BF)
    wo_t = _tile_lhsT(WO).astype(BF)
    w1_t = _tile_lhsT(W1f).astype(BF)
    w2_t = _tile_lhsT(W2).astype(BF)

    def cols(v, nt):
        return c32(np.reshape(v, (nt, 128)).T)

    tri = np.tril(np.ones((128, 128), np.float32)).T  # tri[k, q] = k <= q
    tri2 = np.repeat(tri[:, None, :], 2, axis=1)

    common = dict(
        wq=wq_t, wk=wk_t, wv=wv_r, wo=wo_t, w1=w1_t, w2=w2_t,
        bqc=cols(bQf, CT), bkc=cols(bKf, CT), b1c=cols(b1f, FT),
        boeffc=cols(boeff, CT), b2c=cols(b2, CT),
        onesrv=np.ones((1, 128), np.float32),
        invdrv=np.full((1, 128), 1.0 / D, np.float32),
        tri=tri2.astype(BF),
    )

    in_maps = []
    for core in range(2 * Bn):
        b, p = core // 2, core % 2
        own_rows = np.concatenate(
            [np.arange(128 * (p + 2 * k), 128 * (p + 2 * k) + 128)
             for k in range(n_own)])
        other_rows = np.concatenate(
            [np.arange(128 * ((1 - p) + 2 * k), 128 * ((1 - p) + 2 * k) + 128)
             for k in range(n_own)])
        perm = np.concatenate([own_rows, other_rows])
        m = dict(common)
        m["xt"] = _xT_tiles(X[b][perm], CT).astype(BF)
        m["xot"] = _xT_tiles(X[b][own_rows], CT).astype(np.float32)
        m["pblk"] = np.full((128, 2, 128), float(p == 1), np.float32).astype(BF)
        in_maps.append(m)
    return in_maps


def gather(results, L=L_, D=D_, Bn=B_):
    n_own = (L // 128) // 2
    out = np.empty((Bn, L, D), np.float32)
    for core, r in enumerate(results):
        b, p = core // 2, core % 2
        part = np.ascontiguousarray(r["outT"].T)
        for k in range(n_own):
            out[b, 128 * (p + 2 * k):128 * (p + 2 * k) + 128, :] = \
                part[128 * k:128 * (k + 1), :]
    return out


_NC_CACHE = {}


def get_nc():
    if "nc" not in _NC_CACHE:
        _NC_CACHE["nc"] = build_nc()
    return _NC_CACHE["nc"]


def kernel(**inputs) -> np.ndarray:
    nc, _ = get_nc()
    in_maps = prep_in_maps(inputs)
    res = run_bass_kernel_spmd(nc, in_maps, list(range(N_CORES)))
    return gather(res.results)


# revision 22
# speedup vs baseline: 1.0630x; 1.0630x over previous
"""Trainium2 Bass kernel for a dense transformer block (causal attn + MLP).

Problem: B=4, L=2048, D=1024, H=16 (DH=64), DFF=4096, fp32 in/out.

Sharding (no collectives): 8 cores = 4 batches x 2 parity groups.
Core c handles batch b=c//2 and query-row tiles {p, p+2, ..., p+14}
(p=c%2). The host hands each core its batch TRANSPOSED (feature-major)
and PERMUTED: own sequence tiles first [0:1024), the pair's tiles after
[1024:2048). In permuted space the causal structure is parity-uniform:
own-key tiles hit a STATIC 128x128 triangle mask exactly on the
diagonal chunk, other-key tiles need only an all-0/all-1 per-parity
constant on their boundary chunk - no per-tile mask tables, and Q/K/V,
attention, and the MLP all address compile-time column ranges.

LayerNorm1 runs directly in the transposed layout: an all-1/D bf16
stationary matrix makes the PE stats matmuls produce mean / E[x^2]
pre-broadcast to all 128 partitions (no transposes, no single-partition
ops); ACT squares feed the second-moment accumulation. Q/K/V
projections run as fp8 DoubleRow matmuls (weights + normalized
activations in fp8e4). Scores stay bf16 (K^T/Q^T tiles); exp/V/AV run
in fp8. Softmax normalization is deferred through the AV matmul via
ones columns packed into V; 1/sumexp comes from one fast-approx DVE
reciprocal over the packed sumexp row and a rank-1 PE broadcast. AV
stays in SBUF (bf16) straight into the WO matmul. LN2 reuses the
spread-stats trick, merged into the WO/residual loop. W1 prefetches
during C/D.

dtypes: fp8e4 for QKV weights+activations, V, exp; bf16 for K^T/Q^T,
AV, WO/W1/W2, h, LN2-normalized activations; residual stream fp32(r).
"""

import numpy as np
import ml_dtypes

import concourse.bacc as bacc
import concourse.bass as bass
import concourse.mybir as mybir
import concourse.tile as tile
from concourse.bass_utils import run_bass_kernel_spmd

F32 = mybir.dt.float32
F32R = mybir.dt.float32r
BF16 = mybir.dt.bfloat16
FP8 = mybir.dt.float8e4
BF = ml_dtypes.bfloat16
F8 = ml_dtypes.float8_e4m3fn
EPS = 1e-5
AF = mybir.ActivationFunctionType
OP = mybir.AluOpType
DRM = mybir.MatmulPerfMode.DoubleRow

B_, L_, D_, H_, DFF_ = 4, 2048, 1024, 16, 4096
N_CORES = 8


def _derived(L, D, H, DFF):
    CT = D // 128
    FT = DFF // 128
    n_lt = L // 128
    n_own = n_lt // 2
    OWN_L = n_own * 128
    assert n_own % 4 == 0
    NB = n_own // 4
    HT = H // 2
    assert CT == HT
    VW = min(512, D)
    return dict(CT=CT, FT=FT, n_lt=n_lt, n_own=n_own, OWN_L=OWN_L, NB=NB,
                HT=HT, VW=VW, DVB=D // VW)


def build_nc(L=L_, D=D_, H=H_, DFF=DFF_, n_cores=N_CORES):
    g = _derived(L, D, H, DFF)
    CT, FT = g["CT"], g["FT"]
    n_lt, n_own, OWN_L = g["n_lt"], g["n_own"], g["OWN_L"]
    NB = g["NB"]
    HT, VW, DVB = g["HT"], g["VW"], g["DVB"]
    W = 512
    NBLK = L // W  # A-phase column blocks (permuted; blocks 0..NBLK/2-1 own)
    scale = 1.0 / 8.0  # 1/sqrt(DH)

    nc = bacc.Bacc("TRN2", target_bir_lowering=False, debug=False,
                   num_devices=n_cores)

    dp = nc.declare_dram_parameter
    xt_d = dp("xt", [128, CT, L], BF16, isOutput=False)     # X^T permuted
    xot_d = dp("xot", [128, CT, OWN_L], F32, isOutput=False)  # own X^T exact
    wq_d = dp("wq", [128, CT, CT, 128], FP8, isOutput=False)  # [p, d, c, q]
    wk_d = dp("wk", [128, CT, CT, 128], FP8, isOutput=False)
    wv_d = dp("wv", [128, CT, D], BF16, isOutput=False)        # [p, c, dv]
    wo_d = dp("wo", [CT, 128, CT, 128], BF16, isOutput=False)  # [e, p, c, q]
    w1_d = dp("w1", [FT, 128, CT, 128], BF16, isOutput=False)  # [f, p, c, q]
    w2_d = dp("w2", [CT, 128, FT, 128], BF16, isOutput=False)  # [e, p, f, q]
    bq_d = dp("bqc", [128, CT], F32, isOutput=False)
    bk_d = dp("bkc", [128, CT], F32, isOutput=False)
    b1_d = dp("b1c", [128, FT], F32, isOutput=False)
    boeff_d = dp("boeffc", [128, CT], F32, isOutput=False)
    b2_d = dp("b2c", [128, CT], F32, isOutput=False)
    onesrv_d = dp("onesrv", [1, 128], F32, isOutput=False)
    invdrv_d = dp("invdrv", [1, 128], F32, isOutput=False)
    tri_d = dp("tri", [128, 2, 128], BF16, isOutput=False)
    pblk_d = dp("pblk", [128, 2, 128], BF16, isOutput=False)
    out_d = dp("outT", [D, OWN_L], F32, isOutput=True)

    with tile.TileContext(nc) as tc, \
         nc.allow_low_precision(reason="fp8/f32r/bf16 matmul operands"):
        consts_cm = tc.tile_pool(name="consts", bufs=1)
        consts = consts_cm.__enter__()

        eps_c = consts.tile([128, 1], F32, tag="eps")
        nc.vector.memset(eps_c[:], EPS)
        invd_bf = consts.tile([128, 128], BF16, tag="invdbf")
        nc.vector.memset(invd_bf[:], 1.0 / D)
        _iap = invdrv_d[:]
        invd_m = consts.tile([128, 128], F32R, tag="invdm")
        nc.sync.dma_start(out=invd_m[:], in_=bass.AP(
            tensor=_iap.tensor, offset=_iap.offset,
            ap=[[0, 128], [1, 128]]).bitcast(F32R))
        _oap = onesrv_d[:]
        onesm = consts.tile([128, 128], F32R, tag="onesm")
        nc.sync.dma_start(out=onesm[:], in_=bass.AP(
            tensor=_oap.tensor, offset=_oap.offset,
            ap=[[0, 128], [1, 128]]).bitcast(F32R))
        tri2 = consts.tile([128, 2, 128], BF16, tag="tri2")
        nc.sync.dma_start(out=tri2[:], in_=tri_d[:])
        pblk2 = consts.tile([128, 2, 128], BF16, tag="pblk2")
        nc.sync.dma_start(out=pblk2[:], in_=pblk_d[:])
        bq_sb = consts.tile([128, CT], F32, tag="bq")
        nc.sync.dma_start(out=bq_sb[:], in_=bq_d[:])
        bk_sb = consts.tile([128, CT], F32, tag="bk")
        nc.sync.dma_start(out=bk_sb[:], in_=bk_d[:])
        b1_sb = consts.tile([128, FT], F32, tag="b1")
        nc.sync.dma_start(out=b1_sb[:], in_=b1_d[:])
        boeff_sb = consts.tile([128, CT], F32, tag="boeff")
        nc.sync.dma_start(out=boeff_sb[:], in_=boeff_d[:])
        b2_sb = consts.tile([128, CT], F32, tag="b2")
        nc.sync.dma_start(out=b2_sb[:], in_=b2_d[:])

        # avsb spans B..C (LIFO pool order: open early)
        avsb_cm = tc.tile_pool(name="avsb", bufs=1)
        p_avsb = avsb_cm.__enter__()
        avsb = [p_avsb.tile([128, OWN_L], BF16, tag=f"av{i}", name=f"av{i}")
                for i in range(CT)]
        woP_cm = tc.tile_pool(name="woP", bufs=1)
        woP = woP_cm.__enter__()
        wo_sb = []
        for ei in range(CT):
            wt = woP.tile([128, CT, 128], BF16, tag=f"wo_lhsT{ei}",
                          name=f"wo_lhsT{ei}")
            nc.sync.dma_start(out=wt[:], in_=wo_d[ei])
            wo_sb.append(wt)

        attio_cm = tc.tile_pool(name="attio", bufs=1)
        attio = attio_cm.__enter__()
        kt = [attio.tile([128, L], BF16, tag=f"kt{i}", name=f"kt{i}")
              for i in range(CT)]
        qt = [attio.tile([128, OWN_L], BF16, tag=f"qt{i}", name=f"qt{i}")
              for i in range(CT)]
        v_sb = [attio.tile([128, H, 65], BF16, tag=f"v{i}", name=f"v{i}")
                for i in range(n_lt)]

        # ======== Phase A (LN1+V) then fused per-head K/Q + attention ====
        xn8p_cm = tc.tile_pool(name="xn8p", bufs=1)
        xn8p = xn8p_cm.__enter__()
        xn8s = [xn8p.tile([128, CT, W], FP8, tag=f"xn8_{b}",
                          name=f"xn8_{b}") for b in range(NBLK)]
        wvP_cm = tc.tile_pool(name="wvP", bufs=1)
        wvP = wvP_cm.__enter__()
        wv_sb = wvP.tile([128, CT, D], BF16, tag="wv", name="wv_sb")
        nc.sync.dma_start(out=wv_sb[:], in_=wv_d[:])
        wA_cm = tc.tile_pool(name="workA", bufs=2)
        wA = wA_cm.__enter__()
        psA_st_cm = tc.tile_pool(name="psA_st", bufs=2, space="PSUM")
        psA_st = psA_st_cm.__enter__()
        psA_mm_cm = tc.tile_pool(name="psA_mm", bufs=4, space="PSUM")
        psA_mm = psA_mm_cm.__enter__()

        def ln_blk(blk):
            """LN1 of one 512-col block, fully in the transposed layout.
            Stats matmuls with the all-1/D stationary operand produce
            mean / E[x^2] broadcast to every partition."""
            xt = wA.tile([128, CT, W], BF16, tag="xt", bufs=2)
            nc.sync.dma_start(out=xt[:],
                              in_=xt_d[:, :, blk * W:(blk + 1) * W])
            sqs = []
            for ci in range(CT):
                sq = wA.tile([128, W], BF16, tag="sq", bufs=4)
                nc.scalar.activation(out=sq[:], in_=xt[:, ci, :],
                                     func=AF.Square)
                sqs.append(sq)
            ps_mu = psA_st.tile([128, W], F32, tag="ps_mu")
            ps_sq = psA_st.tile([128, W], F32, tag="ps_sq")
            for ci in range(CT):
                nc.tensor.matmul(ps_mu[:], invd_bf[:], xt[:, ci, :],
                                 start=(ci == 0), stop=(ci == CT - 1))
            for ci in range(CT):
                nc.tensor.matmul(ps_sq[:], invd_bf[:], sqs[ci][:],
                                 start=(ci == 0), stop=(ci == CT - 1))
            mu2 = wA.tile([128, W], F32, tag="mu2", bufs=1)
            nc.vector.tensor_copy(out=mu2[:], in_=ps_mu[:])
            varr = wA.tile([128, W], F32, tag="varr", bufs=1)
            nc.vector.tensor_mul(varr[:], mu2[:], mu2[:])
            nc.vector.tensor_sub(varr[:], ps_sq[:], varr[:])
            std = wA.tile([128, W], F32, tag="std", bufs=1)
            nc.scalar.activation(out=std[:], in_=varr[:], func=AF.Sqrt,
                                 bias=eps_c[:])
            rstd = wA.tile([128, W], F32, tag="rstd", bufs=1)
            nc.vector.reciprocal_approx_fast(out=rstd[:], in_=std[:])
            xnb = wA.tile([128, CT, W], BF16, tag="xnb")
            for ci in range(CT):
                t1 = wA.tile([128, W], F32, tag="t1")
                nc.vector.tensor_sub(t1[:], xt[:, ci, :], ps_mu[:])
                nc.vector.tensor_mul(xnb[:, ci, :], t1[:], rstd[:])
                nc.vector.tensor_copy(out=xn8s[blk][:, ci, :],
                                      in_=xnb[:, ci, :])
            return xnb

        for blk in range(NBLK):
            xnb = ln_blk(blk)
            for st4 in range(4):
                st = 4 * blk + st4
                nc.vector.memset(v_sb[st][:], 1.0)
                for vb in range(DVB):
                    ps = psA_mm.tile([128, VW], F32, tag="ps_mm")
                    for ci in range(CT):
                        nc.tensor.matmul(
                            ps[:],
                            xnb[:, ci, st4 * 128:(st4 + 1) * 128],
                            wv_sb[:, ci, vb * VW:(vb + 1) * VW],
                            start=(ci == 0), stop=(ci == CT - 1))
                    nhh = VW // 64
                    nc.vector.tensor_copy(
                        out=v_sb[st][:, vb * nhh:(vb + 1) * nhh, 0:64],
                        in_=ps[:].rearrange("p (h d) -> p h d", d=64))

        for cm in (psA_mm_cm, psA_st_cm, wA_cm, wvP_cm):
            cm.__exit__(None, None, None)

        # ====== fused per-head-pair K/Q projection + attention ==========
        wqkv_cm = tc.tile_pool(name="wqkv", bufs=1)
        wqkv = wqkv_cm.__enter__()
        wq_sb = wqkv.tile([128, CT, CT, 128], FP8, tag="wq", name="wq_sb")
        nc.sync.dma_start(out=wq_sb[:], in_=wq_d[:])
        wk_sb = wqkv.tile([128, CT, CT, 128], FP8, tag="wk", name="wk_sb")
        nc.sync.dma_start(out=wk_sb[:], in_=wk_d[:])
        wB_cm = tc.tile_pool(name="workB", bufs=4)
        wB = wB_cm.__enter__()
        wR_cm = tc.tile_pool(name="rec", bufs=2)
        wR = wR_cm.__enter__()
        psB_sc_cm = tc.tile_pool(name="psB_sc", bufs=2, space="PSUM")
        psB_sc = psB_sc_cm.__enter__()
        psB_av_cm = tc.tile_pool(name="psB_av", bufs=2, space="PSUM")
        psB_av = psB_av_cm.__enter__()

        def norm_tail(st):
            """Softmax-normalize block (ht,Bk): fast-approx reciprocal of
            the packed sumexp row, PE rank-1 broadcast, DVE multiply into
            avsb. Emitted during the NEXT (ht,Bk) iteration so its latency
            chain never stalls the PE."""
            t_ht, t_Bk, t_av = st
            se = wR.tile([65, 2 * W], F32R, tag="se", name="se")
            for hp in range(2):
                nc.vector.tensor_copy(out=se[64:65, hp * W:(hp + 1) * W],
                                      in_=t_av[hp][64:65, :])
            for hp in range(2):
                ps_bc = psB_sc.tile([128, 2 * W], F32, tag="ps_sc",
                                    name="bc")
                nc.tensor.matmul(ps_bc[0:64, 0:W],
                                 onesm[64:65, 0:64],
                                 se[64:65, hp * W:(hp + 1) * W],
                                 start=True, stop=True)
                bcse = wR.tile([64, W], F32, tag=f"bcse{hp}",
                               name=f"bcse{hp}")
                nc.vector.tensor_copy(out=bcse[:], in_=ps_bc[0:64, 0:W])
                recb = wR.tile([64, W], F32, tag=f"recb{hp}",
                               name=f"recb{hp}")
                nc.vector.reciprocal_approx_fast(out=recb[:], in_=bcse[:])
                nc.vector.tensor_mul(
                    avsb[t_ht][64 * hp:64 * hp + 64,
                               t_Bk * W:(t_Bk + 1) * W],
                    t_av[hp][0:64, :], recb[:])

        def kq_for_ht(ht):
            """K^T (all 4 blocks) and Q^T (2 own blocks) for head pair ht,
            emitted right before this head pair's attention. PSUM comes
            from the score-tile rotation."""
            for qk, wsb, nblk, ktile, bias in (
                    (0, wk_sb, NBLK, kt[ht], bk_sb),
                    (1, wq_sb, NBLK // 2, qt[ht], bq_sb)):
                for blk in range(nblk):
                    half = blk % 2
                    if half == 0:
                        kq_ps = psB_sc.tile([128, 2 * W], F32, tag="ps_sc",
                                            name="kq")
                    for c2 in range(CT // 2):
                        nc.tensor.matmul(
                            kq_ps[:, half * W:(half + 1) * W],
                            wsb[:, ht, 2 * c2:2 * c2 + 2, :],
                            xn8s[blk][:, 2 * c2:2 * c2 + 2, :],
                            perf_mode=DRM, start=(c2 == 0),
                            stop=(c2 == CT // 2 - 1))
                    nc.vector.tensor_scalar_add(
                        out=ktile[:, blk * W:(blk + 1) * W],
                        in0=kq_ps[:, half * W:(half + 1) * W],
                        scalar1=bias[:, ht:ht + 1])

        pending = None
        for ht in range(HT):
            kq_for_ht(ht)
            for Bk in range(NB):
                nt = 4 * Bk + 4
                units = []
                for t in range(nt):  # own-group key tiles
                    units.append((t, 128 * max(0, t - 4 * Bk),
                                  tri2 if t >= 4 * Bk else None))
                for t in range(nt):  # other-group key tiles
                    units.append((8 + t, 128 * max(0, t - 4 * Bk),
                                  pblk2 if t >= 4 * Bk else None))
                n_s = len(units)
                ps_av = [psB_av.tile([128, W], F32, tag=f"ps_av{hp}",
                                     name=f"ps_av{hp}") for hp in range(2)]

                def av_pair(ui, pi, pex, plo):
                    for hp in range(2):
                        nc.tensor.matmul(
                            ps_av[hp][0:65, plo:W],
                            v_sb[pi][:, 2 * ht + hp, :],
                            pex[:, hp * W + plo:(hp + 1) * W],
                            start=(ui == 0), stop=(ui == n_s - 1))

                prevs = []
                for ui, (pi, lo, mk) in enumerate(units):
                    ps_sc = psB_sc.tile([128, 2 * W], F32, tag="ps_sc",
                                        name="ps_sc")
                    for hp in range(2):
                        nc.tensor.matmul(
                            ps_sc[:, hp * W + lo:(hp + 1) * W],
                            kt[ht][64 * hp:64 * hp + 64,
                                   pi * 128:(pi + 1) * 128],
                            qt[ht][64 * hp:64 * hp + 64,
                                   Bk * W + lo:(Bk + 1) * W],
                            start=True, stop=True)
                    ex = wB.tile([128, 2 * W], BF16, tag="exp", name="ex")
                    ex3 = ex[:].rearrange("p (h w) -> p h w", h=2)
                    nc.scalar.activation(
                        out=ex3[:, :, lo:W],
                        in_=ps_sc[:].rearrange("p (h w) -> p h w",
                                               h=2)[:, :, lo:W],
                        func=AF.Exp, scale=scale)
                    if mk is not None:
                        nc.vector.tensor_mul(ex3[:, :, lo:lo + 128],
                                             ex3[:, :, lo:lo + 128],
                                             mk[:])
                    # two-deep software pipeline: AV of unit ui-2 runs on
                    # the PE while exp/mask of ui-1/ui are still on ACT/DVE
                    prevs.append((ui, pi, ex, lo))
                    if len(prevs) > 3:
                        av_pair(*prevs.pop(0))
                    if ui == 2 and pending is not None:
                        norm_tail(pending)
                        pending = None
                for pv in prevs:
                    av_pair(*pv)
                pending = (ht, Bk, ps_av)
        norm_tail(pending)

        for cm in (psB_av_cm, psB_sc_cm, wR_cm, wB_cm, wqkv_cm, xn8p_cm):
            cm.__exit__(None, None, None)
        attio_cm.__exit__(None, None, None)

        # ====== long-lived pools open early: x2t (C..E), x2nt (D->E) and
        # the W1 prefetch ring (DMA spans C/D) =======
        x2t_cm = tc.tile_pool(name="x2t", bufs=1)
        p_x2t = x2t_cm.__enter__()
        x2t = [p_x2t.tile([128, OWN_L], F32R, tag=f"x2t{i}", name=f"x2t{i}")
               for i in range(CT)]
        x2nt_cm = tc.tile_pool(name="x2nt", bufs=1)
        p_x2nt = x2nt_cm.__enter__()
        x2nt = [p_x2nt.tile([128, OWN_L], BF16, tag=f"x2nt{i}",
                            name=f"x2nt{i}") for i in range(CT)]
        wE_cm = tc.tile_pool(name="workE", bufs=2)
        wE = wE_cm.__enter__()
        w1p_cm = tc.tile_pool(name="w1p", bufs=8)
        w1p = w1p_cm.__enter__()

        NPRE = 8
        w1t = {}
        for f in range(NPRE):
            wt = w1p.tile([128, CT, 128], BF16, tag="w1_lhsT")
            nc.sync.dma_start(out=wt[:], in_=w1_d[f])
            w1t[f] = wt

        # ========= Phase C+D: WO proj + residual + LN2 (merged) =========
        xtC_cm = tc.tile_pool(name="xtC", bufs=1)
        p_xtC = xtC_cm.__enter__()
        xt_own = [p_xtC.tile([128, OWN_L], F32, tag=f"xto{i}", name=f"xto{i}")
                  for i in range(CT)]
        wC_cm = tc.tile_pool(name="workC", bufs=2)
        wC = wC_cm.__enter__()
        wD_cm = tc.tile_pool(name="workD", bufs=2)
        wD = wD_cm.__enter__()
        psC_cm = tc.tile_pool(name="psC", bufs=3, space="PSUM")
        psC = psC_cm.__enter__()
        psD_cm = tc.tile_pool(name="psD", bufs=1, space="PSUM")
        psD = psD_cm.__enter__()

        for ci in range(CT):
            nc.sync.dma_start(out=xt_own[ci][:], in_=xot_d[:, ci, :])

        def phase_c(nb):
            for ei in range(CT):
                ps = psC.tile([128, W], F32, tag="ps_o")
                for ci in range(CT):
                    nc.tensor.matmul(ps[:], wo_sb[ei][:, ci, :],
                                     avsb[ci][:, nb * W:(nb + 1) * W],
                                     start=(ci == 0), stop=(ci == CT - 1))
                nc.vector.scalar_tensor_tensor(
                    out=x2t[ei][:, nb * W:(nb + 1) * W], in0=ps[:],
                    scalar=boeff_sb[:, ei:ei + 1],
                    in1=xt_own[ei][:, nb * W:(nb + 1) * W],
                    op0=OP.add, op1=OP.add)

        def phase_d_sq(nb):
            """ACT squares for LN2 stats of block nb (emitted early so the
            PE never waits on them)."""
            sqs = []
            for ci in range(CT):
                sq = wD.tile([128, W], F32R, tag="sq", bufs=8)
                nc.scalar.activation(out=sq[:],
                                     in_=x2t[ci][:, nb * W:(nb + 1) * W],
                                     func=AF.Square)
                sqs.append(sq)
            return sqs

        def phase_d(nb, sqs):
            """LN2 of block nb in transposed layout (spread-stats trick)."""
            ps_mu = psD.tile([128, W], F32, tag="ps_mu")
            ps_sq = psD.tile([128, W], F32, tag="ps_sq")
            for ci in range(CT):
                nc.tensor.matmul(ps_mu[:], invd_m[:],
                                 x2t[ci][:, nb * W:(nb + 1) * W],
                                 start=(ci == 0), stop=(ci == CT - 1))
            for ci in range(CT):
                nc.tensor.matmul(ps_sq[:], invd_m[:], sqs[ci][:],
                                 start=(ci == 0), stop=(ci == CT - 1))
            mu2 = wD.tile([128, W], F32, tag="mu2")
            nc.vector.tensor_copy(out=mu2[:], in_=ps_mu[:])
            varr = wD.tile([128, W], F32, tag="varr")
            nc.vector.tensor_mul(varr[:], mu2[:], mu2[:])
            nc.vector.tensor_sub(varr[:], ps_sq[:], varr[:])
            std = wD.tile([128, W], F32, tag="std")
            nc.scalar.activation(out=std[:], in_=varr[:], func=AF.Sqrt,
                                 bias=eps_c[:])
            rstd = wD.tile([128, W], F32, tag="rstd")
            nc.vector.reciprocal_approx_fast(out=rstd[:], in_=std[:])
            for ci in range(CT):
                t1 = wD.tile([128, W], F32, tag="t1")
                nc.vector.tensor_sub(t1[:], x2t[ci][:, nb * W:(nb + 1) * W],
                                     ps_mu[:])
                nc.vector.tensor_mul(x2nt[ci][:, nb * W:(nb + 1) * W],
                                     t1[:], rstd[:])

        phase_c(0)
        sqs0 = phase_d_sq(0)
        phase_c(1)
        phase_d(0, sqs0)
        sqs1 = phase_d_sq(1)
        phase_d(1, sqs1)

        for cm in (psD_cm, psC_cm, wD_cm, wC_cm, xtC_cm):
            cm.__exit__(None, None, None)

        # ================= Phase E: MLP =================================
        ht_cm = tc.tile_pool(name="hpool", bufs=1)
        p_ht = ht_cm.__enter__()
        h_sb = [p_ht.tile([128, OWN_L], BF16, tag=f"h{i}", name=f"h{i}")
                for i in range(FT)]
        psE_cm = tc.tile_pool(name="psE", bufs=4, space="PSUM")
        psE = psE_cm.__enter__()

        for f in range(FT):
            if f in w1t:
                wtile = w1t.pop(f)
            else:
                wtile = w1p.tile([128, CT, 128], BF16, tag="w1_lhsT")
                nc.sync.dma_start(out=wtile[:], in_=w1_d[f])
            for nb in range(NB):
                ps = psE.tile([128, W], F32, tag="ps_h")
                for ci in range(CT):
                    nc.tensor.matmul(ps[:], wtile[:, ci, :],
                                     x2nt[ci][:, nb * W:(nb + 1) * W],
                                     start=(ci == 0), stop=(ci == CT - 1))
                nc.scalar.activation(out=h_sb[f][:, nb * W:(nb + 1) * W],
                                     in_=ps[:], func=AF.Relu,
                                     bias=b1_sb[:, f:f + 1])
        for ei in range(CT):
            wtile = wE.tile([128, FT, 128], BF16, tag="w2_lhsT")
            nc.sync.dma_start(out=wtile[:], in_=w2_d[ei])
            for nb in range(NB):
                ps = psE.tile([128, W], F32, tag="ps_o2")
                for f in range(FT):
                    nc.tensor.matmul(ps[:], wtile[:, f, :],
                                     h_sb[f][:, nb * W:(nb + 1) * W],
                                     start=(f == 0), stop=(f == FT - 1))
                osb = wE.tile([128, W], F32, tag="osb")
                nc.vector.scalar_tensor_tensor(
                    out=osb[:], in0=ps[:], scalar=b2_sb[:, ei:ei + 1],
                    in1=x2t[ei][:, nb * W:(nb + 1) * W],
                    op0=OP.add, op1=OP.add)
                nc.sync.dma_start(
                    out=out_d[ei * 128:(ei + 1) * 128, nb * W:(nb + 1) * W],
                    in_=osb[:])

        for cm in (psE_cm, ht_cm, w1p_cm, wE_cm, x2nt_cm, x2t_cm, woP_cm,
                   avsb_cm, consts_cm):
            cm.__exit__(None, None, None)

    nc.compile()
    return nc, g


def _tile_lhsT(wmat):
    """[K, M] -> [m, p, c, q] with out[m, p, c, q] = wmat[128c+p, 128m+q]."""
    K, M = wmat.shape
    CT, MT = K // 128, M // 128
    w = wmat.reshape(CT, 128, MT, 128)
    return np.ascontiguousarray(w.transpose(2, 1, 0, 3))


def _fp8(a):
    return np.clip(a, -240.0, 240.0).astype(F8)


def _xT_tiles(Xb, CT):
    """[L?, D] row-major -> [128, CT, L?] feature-major tiles."""
    T = np.ascontiguousarray(Xb.T)  # [D, L]
    Dd, Ln = T.shape
    return np.ascontiguousarray(T.reshape(CT, 128, Ln).transpose(1, 0, 2))


def prep_in_maps(inputs, L=L_, D=D_, H=H_, DFF=DFF_, Bn=B_):
    f64 = lambda k: np.asarray(inputs[k], np.float64)
    X = np.asarray(inputs["X"], np.float32)
    WQ, WK, WV, WO = f64("WQ"), f64("WK"), f64("WV"), f64("WO")
    W1, W2 = f64("W1"), f64("W2")
    bQ, bK, bV, bO = f64("bQ"), f64("bK"), f64("bV"), f64("bO")
    b1, b2 = f64("b1"), f64("b2")
    g1, be1, g2, be2 = f64("g1"), f64("be1"), f64("g2"), f64("be2")

    g = _derived(L, D, H, DFF)
    CT, FT, n_own = g["CT"], g["FT"], g["n_own"]

    # fold LayerNorm affine transforms into the downstream weights
    WQf, bQf = g1[:, None] * WQ, bQ + be1 @ WQ
    WKf, bKf = g1[:, None] * WK, bK + be1 @ WK
    WVf, bVf = g1[:, None] * WV, bV + be1 @ WV
    boeff = bO + WO.T @ bVf
    W1f, b1f = g2[:, None] * W1, b1 + be2 @ W1

    c32 = lambda a: np.ascontiguousarray(a).astype(np.float32)
    wq_t = _fp8(np.ascontiguousarray(
        _tile_lhsT(WQf).transpose(1, 0, 2, 3)))
    wk_t = _fp8(np.ascontiguousarray(
        _tile_lhsT(WKf).transpose(1, 0, 2, 3)))
    wv_r = np.ascontiguousarray(
        WVf.reshape(CT, 128, D).transpose(1, 0, 2)).astype(BF)
    wo_t = _tile_lhsT(WO).astype(BF)
    w1_t = _tile_lhsT(W1f).astype(BF)
    w2_t = _tile_lhsT(W2).astype(BF)

    def cols(v, nt):
        return c32(np.reshape(v, (nt, 128)).T)

    tri = np.tril(np.ones((128, 128), np.float32)).T  # tri[k, q] = k <= q
    tri2 = np.repeat(tri[:, None, :], 2, axis=1)

    common = dict(
        wq=wq_t, wk=wk_t, wv=wv_r, wo=wo_t, w1=w1_t, w2=w2_t,
        bqc=cols(bQf, CT), bkc=cols(bKf, CT), b1c=cols(b1f, FT),
        boeffc=cols(boeff, CT), b2c=cols(b2, CT),
        onesrv=np.ones((1, 128), np.float32),
        invdrv=np.full((1, 128), 1.0 / D, np.float32),
        tri=tri2.astype(BF),
    )

    in_maps = []
    for core in range(2 * Bn):
        b, p = core // 2, core % 2
        own_rows = np.concatenate(
            [np.arange(128 * (p + 2 * k), 128 * (p + 2 * k) + 128)
             for k in range(n_own)])
        other_rows = np.concatenate(
            [np.arange(128 * ((1 - p) + 2 * k), 128 * ((1 - p) + 2 * k) + 128)
             for k in range(n_own)])
        perm = np.concatenate([own_rows, other_rows])
        m = dict(common)
        m["xt"] = _xT_tiles(X[b][perm], CT).astype(BF)
        m["xot"] = _xT_tiles(X[b][own_rows], CT).astype(np.float32)
        m["pblk"] = np.full((128, 2, 128), float(p == 1), np.float32).astype(BF)
        in_maps.append(m)
    return in_maps


def gather(results, L=L_, D=D_, Bn=B_):
    n_own = (L // 128) // 2
    out = np.empty((Bn, L, D), np.float32)
    for core, r in enumerate(results):
        b, p = core // 2, core % 2
        part = np.ascontiguousarray(r["outT"].T)
        for k in range(n_own):
            out[b, 128 * (p + 2 * k):128 * (p + 2 * k) + 128, :] = \
                part[128 * k:128 * (k + 1), :]
    return out


_NC_CACHE = {}


def get_nc():
    if "nc" not in _NC_CACHE:
        _NC_CACHE["nc"] = build_nc()
    return _NC_CACHE["nc"]


def kernel(**inputs) -> np.ndarray:
    nc, _ = get_nc()
    in_maps = prep_in_maps(inputs)
    res = run_bass_kernel_spmd(nc, in_maps, list(range(N_CORES)))
    return gather(res.results)
